# revision 13
# baseline (speedup 1.0000x reference)
"""Distributed Trainium2 kernel for nn_Attention_21990232555717.

Reference (per batch element a, seq s=1024, model dim c=1024, 16 heads):
    qkv = x @ w_qkv                       # (s, 3072)
    q,k,v split per head (hd=64)
    scores = q @ k.T * (1/sqrt(1024))     # (h, s, s)
    attn = softmax(scores, axis=HEADS)    # normalize across the 16 heads!
    out = attn @ v -> (s, 1024) @ w_out + b_out

Sharding: pure data parallel - batch (8) across 8 cores, weights replicated.

v2.1 design:
  * all-bf16 datapath (rel err ~5e-3 vs the 2e-2 gate)
  * weights arrive bf16 via SWDGE cast-DMA: wv first (v_proj runs before
    qk_proj so Vb is ready early), then wqk, then wout
  * the ACT exp stream (128 exps of [128,1024] ~ 147us) is the wall:
    scores for qb0 chase the qk pairs during the projections, and in the
    steady state each qb block interleaves next-group scores so ACT never
    idles.  ACT does ONLY exps + QKT/Vb psum copies.
  * DVE: den chains (accumulate heads 8-15), recip, normalize muls
    (pair-granular against a doubled rec buffer - real strides keep the
    2x_1P mode; a broadcast AP drops to 1x and doubles the cost), attnV
    psum->outT copies, y bias.  All den/mul work for qb is emitted one
    block EARLY so attnV(qb) is never mul-gated.
  * GpSimd: denominator partial sums over heads 0-7 + the leading pair
    adds of the 8-15 chains (own queue, off critical path).
  * attnV accumulates all 8 k-tiles directly in PSUM (no spill/merge) -
    legal because the E ring holds 3 groups: 2 in e_p plus the dead wqk
    slot re-viewed as the third buffer.
  * PSUM: ps_proj [128,1024] bufs=1 (transposes+projections, 2 banks),
    ps_sc [128,1024] bufs=1 (scores, 2 banks), ps_a [128,512] bufs=4
    (attnV waves + out-proj units + bias bcast, 4 banks).
  * SBUF aliasing: wqk slot -> 3rd E buffer; wv slot -> outT scratch;
    xT slot -> wout.
"""

import numpy as np

import concourse.bass as bass
import concourse.mybir as mybir
import concourse.tile as tile
from concourse import bacc
from concourse.bass_utils import run_bass_kernel_spmd
from concourse.masks import make_identity

F32 = mybir.dt.float32
BF16 = mybir.dt.bfloat16
Exp = mybir.ActivationFunctionType.Exp
Bypass = mybir.AluOpType.bypass
Add = mybir.AluOpType.add

S = 1024      # sequence length per core (batch element)
C = 1024      # model dim
H = 16        # heads
HD = 64       # head dim
SCALE = 1.0 / (C ** 0.5)
QB = 256      # q block size
NQB = S // QB          # 4 q blocks
NKT = S // 128         # 8 k tiles
NCT = C // 128         # 8 contraction tiles
NPAIR = 8              # head pairs


def build():
    nc = bacc.Bacc(None, target_bir_lowering=False)
    x_ext = nc.declare_dram_parameter("x", [S, C], F32, isOutput=False)
    wqkv_ext = nc.declare_dram_parameter("w_qkv", [C, 3 * C], F32, isOutput=False)
    wout_ext = nc.declare_dram_parameter("w_out", [C, C], F32, isOutput=False)
    b_ext = nc.declare_dram_parameter("b_out", [C], F32, isOutput=False)
    out_ext = nc.declare_dram_parameter("out", [S, C], F32, isOutput=True)

    with tile.TileContext(nc) as tc:
        with (
            tc.tile_pool(name="const_p", bufs=1) as const_p,
            tc.tile_pool(name="xf_p", bufs=1) as xf_p,
            tc.tile_pool(name="xb_p", bufs=1) as xb_p,
            tc.tile_pool(name="xt_p", bufs=1) as xt_p,      # xT, then wout
            tc.tile_pool(name="w_p", bufs=1) as w_p,        # wqk->E3, wv->outT
            tc.tile_pool(name="act_p", bufs=1) as act_p,
            tc.tile_pool(name="e_p", bufs=2) as e_p,
            tc.tile_pool(name="d_p", bufs=1) as d_p,
            tc.tile_pool(name="r_p", bufs=1) as r_p,
            tc.tile_pool(name="y_p", bufs=2) as y_p,
            tc.tile_pool(name="ps_j", bufs=2, space="PSUM") as ps_j,
            tc.tile_pool(name="ps_s", bufs=1, space="PSUM") as ps_s,
            tc.tile_pool(name="ps_a", bufs=2, space="PSUM") as ps_a,
        ):
            # ---- weights first: the SWDGE queue is the start-latency gate ----
            wqk = w_p.tile([128, NCT, 2 * C], BF16, tag="wqk", name="wqk")  # 32 KB
            wv = w_p.tile([128, NCT, C], BF16, tag="wv", name="wv")         # 16 KB
            for ct in range(NCT):
                nc.gpsimd.dma_start(wv[:, ct, :],
                                    wqkv_ext[ct * 128:(ct + 1) * 128, 2 * C:3 * C])
            for ct in range(NCT):
                nc.gpsimd.dma_start(wqk[:, ct, :],
                                    wqkv_ext[ct * 128:(ct + 1) * 128, 0:2 * C])
            b_sb = const_p.tile([1, C], BF16)
            nc.gpsimd.dma_start(b_sb, b_ext[None, :])

            # ---- constants ----
            ident = const_p.tile([128, 128], BF16)
            make_identity(nc, ident)
            ones1 = const_p.tile([1, 128], BF16)
            nc.vector.memset(ones1, 1.0)

            # ---- persistent activations ----
            QKT = act_p.tile([128, H, S], BF16)        # 32 KB/part (ft 0-7 Q, 8-15 K)
            Vb = act_p.tile([128, NKT, C], BF16)       # 16 KB/part
            b_bcast = act_p.tile([128, C], BF16)       # 2 KB/part

            # ---- x: HWDGE f32 + DVE cast + PE transpose (bf16) ----
            xT = xt_p.tile([128, NCT, S], BF16, tag="xt", name="xT")  # 16 KB
            for st in range(NKT):
                xf = xf_p.tile([128, C], F32, tag="xf", name=f"xf{st}")
                eng = nc.sync if st % 2 == 0 else nc.scalar
                eng.dma_start(xf, x_ext[st * 128:(st + 1) * 128, :])
                xb = xb_p.tile([128, C], BF16, tag="xb", name=f"xb{st}")
                nc.vector.tensor_copy(xb, xf)
                pt = ps_j.tile([128, S], F32, tag="pj", name=f"pt{st}")
                ptb = pt.bitcast(BF16)  # bf16 view: transpose out must be bf16
                for ct in range(NCT):
                    nc.tensor.transpose(ptb[:, ct * 128:(ct + 1) * 128],
                                        xb[:, ct * 128:(ct + 1) * 128], ident)
                ptv = ptb[:, 0:C].rearrange("p (a b) -> p a b", a=NCT)
                nc.vector.tensor_copy(xT[:, :, st * 128:(st + 1) * 128], ptv)

            # b_bcast: broadcast bias to all partitions via ones-matmul
            for ec in range(2):
                psb = ps_a.tile([128, 512], F32, tag="acc", name=f"psbb{ec}")
                nc.tensor.matmul(psb, ones1, b_sb[:, ec * 512:(ec + 1) * 512],
                                 start=True, stop=True)
                nc.vector.tensor_copy(b_bcast[:, ec * 512:(ec + 1) * 512], psb)

            # ================= interleaved main pipeline =================
            Egrp = {}      # (qb, gg) -> E tile view [128, H, 4*QB]
            SLOW = {}      # (qb, gg) -> partial-sum chains tile [128, 2, S]
            REC2 = {}      # (qb, gg) -> doubled-rec tile [128, 2, S] bf16
            # chain storage aliasing (saves SBUF):
            #   sl[:,0,:]  heads 0-3 then 0-7 sum  (GpSimd)
            #   sl[:,1,:]  heads 4-7 sum (GpSimd), then REUSED as the
            #              heads 8-11 chain (GpSimd pair add + DVE accums -
            #              same-queue order makes the WAR safe)
            #   rec2[:,0,:] heads 12-15 chain, overwritten by rec after its
            #              last read; rec2[:,1,:] = second rec copy

            def emit_qk_ft(ft):
                ps = ps_j.tile([128, S], F32, tag="pj", name=f"qk{ft}")
                for ct in range(NCT):
                    lhsT = wqk[:, ct, ft * 128:(ft + 1) * 128]
                    for sb in range(2):
                        nc.tensor.matmul(
                            ps[:, sb * 512:(sb + 1) * 512], lhsT,
                            xT[:, ct, sb * 512:(sb + 1) * 512],
                            start=(ct == 0), stop=(ct == NCT - 1))
                nc.scalar.copy(QKT[:, ft, :], ps)

            def emit_v_st(st):
                ps = ps_j.tile([128, S], F32, tag="pj", name=f"v{st}")
                for ct in range(NCT):
                    lhsT = xT[:, ct, st * 128:(st + 1) * 128]
                    for fb in range(2):
                        nc.tensor.matmul(
                            ps[:, fb * 512:(fb + 1) * 512], lhsT,
                            wv[:, ct, fb * 512:(fb + 1) * 512],
                            start=(ct == 0), stop=(ct == NCT - 1))
                nc.scalar.copy(Vb[:, st, :], ps)

            def new_group(qb, gg):
                i = 2 * qb + gg
                if i % 3 == 2:
                    # 3rd ring slot: re-view the dead wqk slot (same bytes)
                    ew = w_p.tile([128, NCT, 2 * C], BF16, tag="wqk",
                                  name=f"Ew{i}")
                    Egrp[(qb, gg)] = ew.rearrange("p a (b c) -> p (a b) c",
                                                  b=2, c=1024)
                else:
                    Egrp[(qb, gg)] = e_p.tile([128, H, 4 * QB], BF16, tag="E",
                                              name=f"E{qb}_{gg}")
                SLOW[(qb, gg)] = d_p.tile([128, 2, 4 * QB], BF16, tag="slow",
                                          name=f"sl{qb}_{gg}")
                REC2[(qb, gg)] = r_p.tile([128, 2, 4 * QB], BF16, tag="rec2",
                                          name=f"rec2{qb}_{gg}")

            def emit_scores(qb, gg, h):
                """scores + exp + gpsimd denominator links for head h."""
                E = Egrp[(qb, gg)]
                sl = SLOW[(qb, gg)]
                q0 = qb * QB
                po = 64 * (h % 2)
                rhs = QKT[po:po + 64, h // 2, q0:q0 + QB]
                pss = ps_s.tile([128, 4 * QB], F32, tag="ps",
                                name=f"sc{qb}_{gg}_{h}")
                for j in range(4):
                    kt = 4 * gg + j
                    lhsT = QKT[po:po + 64, 8 + h // 2, kt * 128:(kt + 1) * 128]
                    nc.tensor.matmul(pss[:, j * QB:(j + 1) * QB], lhsT, rhs,
                                     start=True, stop=True)
                nc.scalar.activation(E[:, h, :], pss, Exp, scale=SCALE)
                # gpsimd tree over heads 0-7:
                #   sl0 = E0+E1 (+E2) (+E3); sl1 = E4+E5 (+E6) (+E7); sl0 += sl1
                if h < 8 and h % 2 == 1:
                    c = h // 4
                    if h % 4 == 1:
                        nc.gpsimd.tensor_add(sl[:, c, :], E[:, h - 1, :],
                                             E[:, h, :])
                    else:
                        nc.gpsimd.tensor_add(sl[:, c, :], sl[:, c, :],
                                             E[:, h - 1, :])
                        nc.gpsimd.tensor_add(sl[:, c, :], sl[:, c, :],
                                             E[:, h, :])
                        if h == 7:
                            nc.gpsimd.tensor_add(sl[:, 0, :], sl[:, 0, :],
                                                 sl[:, 1, :])
                # gpsimd also does the leading pair adds of the 8-15 chains
                elif h == 9:
                    sl2 = SLOW[(qb, gg)]
                    nc.gpsimd.tensor_add(sl2[:, 1, :], E[:, 8, :], E[:, 9, :])
                elif h == 13:
                    r2 = REC2[(qb, gg)]
                    nc.gpsimd.tensor_add(r2[:, 0, :], E[:, 12, :], E[:, 13, :])

            def emit_den_dve(qb, gg):
                """DVE: finish the 8-15 chains, D, recip, doubled rec,
                then the wave-ordered pair muls."""
                E = Egrp[(qb, gg)]
                sl = SLOW[(qb, gg)]
                rec2 = REC2[(qb, gg)]
                ud0 = sl[:, 1, :]       # heads 8-11 chain (seeded on GpSimd)
                ud1 = rec2[:, 0, :]     # heads 12-15 chain (seeded on GpSimd)
                nc.vector.tensor_add(ud0, ud0, E[:, 10, :])
                nc.vector.tensor_add(ud0, ud0, E[:, 11, :])
                nc.vector.tensor_add(ud1, ud1, E[:, 14, :])
                nc.vector.tensor_add(ud1, ud1, E[:, 15, :])
                nc.vector.tensor_add(ud0, ud0, ud1)
                denf = r_p.tile([128, 4 * QB], F32, tag="denf",
                                name=f"denf{qb}_{gg}")
                nc.vector.tensor_add(denf, ud0, sl[:, 0, :])
                nc.vector.reciprocal_approx_fast(out=denf, in_=denf)
                nc.vector.tensor_copy(rec2[:, 0, :], denf)
                nc.vector.tensor_copy(rec2[:, 1, :], denf)
                # wave-ordered pair muls: real strides (no broadcast) keep 2x
                for w in range(NPAIR):
                    nc.vector.tensor_mul(E[:, 2 * w:2 * w + 2, :],
                                         E[:, 2 * w:2 * w + 2, :], rec2)

            def emit_attnv_wave(qb, w, outT):
                """attnV for head pair w over ALL 8 k-tiles, then one copy."""
                aw = ps_a.tile([128, 512], F32, tag="acc", name=f"aw{qb}_{w}")
                for kt in range(NKT):
                    E = Egrp[(qb, kt // 4)]
                    j = kt % 4
                    for i in range(2):
                        h = 2 * w + i
                        po = 64 * (h % 2)
                        nc.tensor.matmul(
                            aw[po:po + 64, 0:QB],
                            Vb[:, kt, h * HD:(h + 1) * HD],
                            E[:, h, j * QB:(j + 1) * QB],
                            start=(kt == 0), stop=(kt == NKT - 1),
                            tile_position=(0, po))
                nc.vector.tensor_copy(outT[:, w, :], aw[:, 0:QB])

            def emit_out_proj_qsub(qb, outT, qsub, wout):
                q0 = qb * QB
                psy = [ps_a.tile([128, 512], F32, tag="acc",
                                 name=f"psy{qb}_{qsub}_{ec}") for ec in range(2)]
                for ft in range(NCT):
                    lhsT = outT[:, ft, qsub * 128:(qsub + 1) * 128]
                    for ec in range(2):
                        nc.tensor.matmul(psy[ec], lhsT,
                                         wout[:, ft, ec * 512:(ec + 1) * 512],
                                         start=(ft == 0), stop=(ft == NCT - 1))
                for ec in range(2):
                    y = y_p.tile([128, 512], F32, tag="y",
                                 name=f"y{qb}_{qsub}_{ec}")
                    nc.vector.scalar_tensor_tensor(
                        y, psy[ec], 0.0, b_bcast[:, ec * 512:(ec + 1) * 512],
                        Bypass, Add)
                    nc.sync.dma_start(
                        out_ext[q0 + qsub * 128:q0 + (qsub + 1) * 128,
                                ec * 512:(ec + 1) * 512], y)

            # ---------- phase A: v_proj, then qk pairs + qb0 scores ----------
            new_group(0, 0)
            new_group(0, 1)
            with nc.named_scope("v_proj"):
                for st in range(NKT):
                    emit_v_st(st)
            with nc.named_scope("qk_attn_overlap"):
                for p in range(NPAIR):
                    emit_qk_ft(8 + p)   # K pair p
                    emit_qk_ft(p)       # Q pair p
                    # scores for the previous pair (1-pair lag keeps PE fed)
                    if p >= 1:
                        for gg in range(2):
                            for i in range(2):
                                emit_scores(0, gg, 2 * (p - 1) + i)
                for gg in range(2):
                    for i in range(2):
                        emit_scores(0, gg, 2 * 7 + i)
                emit_den_dve(0, 0)
                emit_den_dve(0, 1)

            # wout: reuses the xT pool slot (xT dead after qk_proj)
            wout = xt_p.tile([128, NCT, C], BF16, tag="xt", name="wout")
            for ct in range(NCT):
                nc.gpsimd.dma_start(wout[:, ct, :],
                                    wout_ext[ct * 128:(ct + 1) * 128, :])

            # outT scratch: reuses the wv slot (dead after v_proj); even/odd
            # qbs use disjoint column ranges, range-level deps handle reuse.
            scratch = w_p.tile([128, NCT, C], BF16, tag="wv", name="scratch")
            outT_views = [scratch[:, :, 0:QB], scratch[:, :, QB:2 * QB]]

            # ---------- phase B: attention pipeline over qbs ----------
            for qb in range(NQB):
                with nc.named_scope(f"attn_qb{qb}"):
                    outT = outT_views[qb % 2]
                    # next-group scores first: ACT runs ahead into (qb+1, 0)
                    if qb + 1 < NQB:
                        new_group(qb + 1, 0)
                        for h in range(H):
                            emit_scores(qb + 1, 0, h)
                    for w in range(NPAIR):
                        emit_attnv_wave(qb, w, outT)
                    # E(qb,*) free -> scores for (qb+1, 1)
                    if qb + 1 < NQB:
                        new_group(qb + 1, 1)
                        for h in range(H):
                            emit_scores(qb + 1, 1, h)
                    emit_out_proj_qsub(qb, outT, 0, wout)
                    emit_out_proj_qsub(qb, outT, 1, wout)
                    # den/mul work for the NEXT qb, emitted now so attnV(qb+1)
                    # is never mul-gated; these wait on (qb+1)'s exps, so they
                    # go LAST in the DVE queue.
                    if qb + 1 < NQB:
                        emit_den_dve(qb + 1, 0)
                        emit_den_dve(qb + 1, 1)

    nc.compile()
    return nc


_NC = None


def _get_nc():
    global _NC
    if _NC is None:
        _NC = build()
    return _NC


def kernel(x, w_qkv, w_out, b_out):
    nc = _get_nc()
    x = np.ascontiguousarray(np.asarray(x, dtype=np.float32))
    w_qkv = np.ascontiguousarray(np.asarray(w_qkv, dtype=np.float32))
    w_out = np.ascontiguousarray(np.asarray(w_out, dtype=np.float32))
    b_out = np.ascontiguousarray(np.asarray(b_out, dtype=np.float32))
    in_maps = [
        {"x": x[i], "w_qkv": w_qkv, "w_out": w_out, "b_out": b_out}
        for i in range(8)
    ]
    res = run_bass_kernel_spmd(nc, in_maps, core_ids=list(range(8)))
    out = np.stack([np.asarray(res.results[i]["out"]) for i in range(8)])
    return out.astype(np.float32)


# revision 18
# speedup vs baseline: 1.0257x; 1.0257x over previous
"""Distributed Trainium2 kernel for nn_Attention_21990232555717.

Reference (per batch element a, seq s=1024, model dim c=1024, 16 heads):
    qkv = x @ w_qkv                       # (s, 3072)
    q,k,v split per head (hd=64)
    scores = q @ k.T * (1/sqrt(1024))     # (h, s, s)
    attn = softmax(scores, axis=HEADS)    # normalize across the 16 heads!
    out = attn @ v -> (s, 1024) @ w_out + b_out

Sharding: pure data parallel - batch (8) across 8 cores, weights replicated.

v2.1 design:
  * all-bf16 datapath (rel err ~5e-3 vs the 2e-2 gate)
  * weights arrive bf16 via SWDGE cast-DMA: wv first (v_proj runs before
    qk_proj so Vb is ready early), then wqk, then wout
  * the ACT exp stream (128 exps of [128,1024] ~ 147us) is the wall:
    scores for qb0 chase the qk pairs during the projections, and in the
    steady state each qb block interleaves next-group scores so ACT never
    idles.  ACT does ONLY exps + QKT/Vb psum copies.
  * DVE: den chains (accumulate heads 8-15), recip, normalize muls
    (pair-granular against a doubled rec buffer - real strides keep the
    2x_1P mode; a broadcast AP drops to 1x and doubles the cost), attnV
    psum->outT copies, y bias.  All den/mul work for qb is emitted one
    block EARLY so attnV(qb) is never mul-gated.
  * GpSimd: denominator partial sums over heads 0-7 + the leading pair
    adds of the 8-15 chains (own queue, off critical path).
  * attnV accumulates all 8 k-tiles directly in PSUM (no spill/merge) -
    legal because the E ring holds 3 groups: 2 in e_p plus the dead wqk
    slot re-viewed as the third buffer.
  * PSUM: ps_proj [128,1024] bufs=1 (transposes+projections, 2 banks),
    ps_sc [128,1024] bufs=1 (scores, 2 banks), ps_a [128,512] bufs=4
    (attnV waves + out-proj units + bias bcast, 4 banks).
  * SBUF aliasing: wqk slot -> 3rd E buffer; wv slot -> outT scratch;
    xT slot -> wout.
"""

import numpy as np

import concourse.bass as bass
import concourse.mybir as mybir
import concourse.tile as tile
from concourse import bacc
from concourse.bass_utils import run_bass_kernel_spmd
from concourse.masks import make_identity

F32 = mybir.dt.float32
BF16 = mybir.dt.bfloat16
Exp = mybir.ActivationFunctionType.Exp
Bypass = mybir.AluOpType.bypass
Add = mybir.AluOpType.add

S = 1024      # sequence length per core (batch element)
C = 1024      # model dim
H = 16        # heads
HD = 64       # head dim
SCALE = 1.0 / (C ** 0.5)
QB = 256      # q block size
NQB = S // QB          # 4 q blocks
NKT = S // 128         # 8 k tiles
NCT = C // 128         # 8 contraction tiles
NPAIR = 8              # head pairs


def build():
    nc = bacc.Bacc(None, target_bir_lowering=False)
    x_ext = nc.declare_dram_parameter("x", [S, C], F32, isOutput=False)
    wqkv_ext = nc.declare_dram_parameter("w_qkv", [C, 3 * C], F32, isOutput=False)
    wout_ext = nc.declare_dram_parameter("w_out", [C, C], F32, isOutput=False)
    b_ext = nc.declare_dram_parameter("b_out", [C], F32, isOutput=False)
    out_ext = nc.declare_dram_parameter("out", [S, C], F32, isOutput=True)

    with tile.TileContext(nc) as tc:
        with (
            tc.tile_pool(name="const_p", bufs=1) as const_p,
            tc.tile_pool(name="xb_p", bufs=2) as xb_p,
            tc.tile_pool(name="xt_p", bufs=1) as xt_p,      # xT, then wout
            tc.tile_pool(name="w_p", bufs=1) as w_p,        # wqk->E3, wv->outT
            tc.tile_pool(name="act_p", bufs=1) as act_p,
            tc.tile_pool(name="e_p", bufs=2) as e_p,
            tc.tile_pool(name="d_p", bufs=1) as d_p,
            tc.tile_pool(name="r_p", bufs=1) as r_p,
            tc.tile_pool(name="y_p", bufs=2) as y_p,
            tc.tile_pool(name="ps_j", bufs=2, space="PSUM") as ps_j,
            tc.tile_pool(name="ps_s", bufs=1, space="PSUM") as ps_s,
            tc.tile_pool(name="ps_a", bufs=2, space="PSUM") as ps_a,
        ):
            # ---- weights first: the SWDGE queue is the start-latency gate ----
            wqk = w_p.tile([128, NCT, 2 * C], BF16, tag="wqk", name="wqk")  # 32 KB
            wv = w_p.tile([128, NCT, C], BF16, tag="wv", name="wv")         # 16 KB
            for ct in range(NCT):
                nc.gpsimd.dma_start(wv[:, ct, :],
                                    wqkv_ext[ct * 128:(ct + 1) * 128, 2 * C:3 * C])
            for ct in range(NCT):
                nc.gpsimd.dma_start(wqk[:, ct, :],
                                    wqkv_ext[ct * 128:(ct + 1) * 128, 0:2 * C])
            b_sb = const_p.tile([1, C], BF16)
            nc.gpsimd.dma_start(b_sb, b_ext[None, :])

            # ---- constants ----
            ident = const_p.tile([128, 128], BF16)
            make_identity(nc, ident)
            ones1 = const_p.tile([1, 128], BF16)
            nc.vector.memset(ones1, 1.0)

            # ---- persistent activations ----
            QKT = act_p.tile([128, H, S], BF16)        # 32 KB/part (ft 0-7 Q, 8-15 K)
            Vb = act_p.tile([128, NKT, C], BF16)       # 16 KB/part
            b_bcast = act_p.tile([128, C], BF16)       # 2 KB/part

            # ---- x: HWDGE f32 DMA staged INSIDE E(0,0) (f32 view; the tile
            # is not written by exps until ~60us) -> DVE cast -> PE transpose.
            # All 8 slab DMAs land in parallel (no ring-latency serialization).
            E00 = e_p.tile([128, H, 4 * QB], BF16, tag="E", name="E0_0")
            xstage = E00.bitcast(F32).rearrange("p a b -> p (a b)")  # [128, 8192] f32
            xT = xt_p.tile([128, NCT, S], BF16, tag="xt", name="xT")  # 16 KB
            for st in range(NKT):
                xf = xstage[:, st * C:(st + 1) * C]
                eng = nc.sync if st % 2 == 0 else nc.scalar
                eng.dma_start(xf, x_ext[st * 128:(st + 1) * 128, :])
                xb = xb_p.tile([128, C], BF16, tag="xb", name=f"xb{st}")
                nc.vector.tensor_copy(xb, xf)
                pt = ps_j.tile([128, S], F32, tag="pj", name=f"pt{st}")
                ptb = pt.bitcast(BF16)  # bf16 view: transpose out must be bf16
                for ct in range(NCT):
                    nc.tensor.transpose(ptb[:, ct * 128:(ct + 1) * 128],
                                        xb[:, ct * 128:(ct + 1) * 128], ident)
                ptv = ptb[:, 0:C].rearrange("p (a b) -> p a b", a=NCT)
                nc.vector.tensor_copy(xT[:, :, st * 128:(st + 1) * 128], ptv)

            # b_bcast: broadcast bias to all partitions via ones-matmul
            for ec in range(2):
                psb = ps_a.tile([128, 512], F32, tag="acc", name=f"psbb{ec}")
                nc.tensor.matmul(psb, ones1, b_sb[:, ec * 512:(ec + 1) * 512],
                                 start=True, stop=True)
                nc.vector.tensor_copy(b_bcast[:, ec * 512:(ec + 1) * 512], psb)

            # ================= interleaved main pipeline =================
            Egrp = {}      # (qb, gg) -> E tile view [128, H, 4*QB]
            SLOW = {}      # (qb, gg) -> partial-sum chains tile [128, 2, S]
            REC2 = {}      # (qb, gg) -> doubled-rec tile [128, 2, S] bf16
            # chain storage aliasing (saves SBUF):
            #   sl[:,0,:]  heads 0-3 then 0-7 sum  (GpSimd)
            #   sl[:,1,:]  heads 4-7 sum (GpSimd), then REUSED as the
            #              heads 8-11 chain (GpSimd pair add + DVE accums -
            #              same-queue order makes the WAR safe)
            #   rec2[:,0,:] heads 12-15 chain, overwritten by rec after its
            #              last read; rec2[:,1,:] = second rec copy

            def emit_qk_ft(ft):
                ps = ps_j.tile([128, S], F32, tag="pj", name=f"qk{ft}")
                for ct in range(NCT):
                    lhsT = wqk[:, ct, ft * 128:(ft + 1) * 128]
                    for sb in range(2):
                        nc.tensor.matmul(
                            ps[:, sb * 512:(sb + 1) * 512], lhsT,
                            xT[:, ct, sb * 512:(sb + 1) * 512],
                            start=(ct == 0), stop=(ct == NCT - 1))
                nc.scalar.copy(QKT[:, ft, :], ps)

            def emit_v_st(st):
                ps = ps_j.tile([128, S], F32, tag="pj", name=f"v{st}")
                for ct in range(NCT):
                    lhsT = xT[:, ct, st * 128:(st + 1) * 128]
                    for fb in range(2):
                        nc.tensor.matmul(
                            ps[:, fb * 512:(fb + 1) * 512], lhsT,
                            wv[:, ct, fb * 512:(fb + 1) * 512],
                            start=(ct == 0), stop=(ct == NCT - 1))
                nc.scalar.copy(Vb[:, st, :], ps)

            def new_group(qb, gg):
                i = 2 * qb + gg
                if i == 0:
                    Egrp[(qb, gg)] = E00  # pre-allocated (doubles as x stage)
                elif i % 3 == 2:
                    # 3rd ring slot: re-view the dead wqk slot (same bytes)
                    ew = w_p.tile([128, NCT, 2 * C], BF16, tag="wqk",
                                  name=f"Ew{i}")
                    Egrp[(qb, gg)] = ew.rearrange("p a (b c) -> p (a b) c",
                                                  b=2, c=1024)
                else:
                    Egrp[(qb, gg)] = e_p.tile([128, H, 4 * QB], BF16, tag="E",
                                              name=f"E{qb}_{gg}")
                SLOW[(qb, gg)] = d_p.tile([128, 2, 4 * QB], BF16, tag="slow",
                                          name=f"sl{qb}_{gg}")
                REC2[(qb, gg)] = r_p.tile([128, 2, 4 * QB], BF16, tag="rec2",
                                          name=f"rec2{qb}_{gg}")

            def emit_scores(qb, gg, h):
                """scores + exp + gpsimd denominator links for head h."""
                E = Egrp[(qb, gg)]
                sl = SLOW[(qb, gg)]
                q0 = qb * QB
                po = 64 * (h % 2)
                rhs = QKT[po:po + 64, h // 2, q0:q0 + QB]
                pss = ps_s.tile([128, 4 * QB], F32, tag="ps",
                                name=f"sc{qb}_{gg}_{h}")
                for j in range(4):
                    kt = 4 * gg + j
                    lhsT = QKT[po:po + 64, 8 + h // 2, kt * 128:(kt + 1) * 128]
                    nc.tensor.matmul(pss[:, j * QB:(j + 1) * QB], lhsT, rhs,
                                     start=True, stop=True)
                nc.scalar.activation(E[:, h, :], pss, Exp, scale=SCALE)
                # gpsimd tree over heads 0-7:
                #   sl0 = E0+E1 (+E2) (+E3); sl1 = E4+E5 (+E6) (+E7); sl0 += sl1
                if h < 8 and h % 2 == 1:
                    c = h // 4
                    if h % 4 == 1:
                        nc.gpsimd.tensor_add(sl[:, c, :], E[:, h - 1, :],
                                             E[:, h, :])
                    else:
                        nc.gpsimd.tensor_add(sl[:, c, :], sl[:, c, :],
                                             E[:, h - 1, :])
                        nc.gpsimd.tensor_add(sl[:, c, :], sl[:, c, :],
                                             E[:, h, :])
                        if h == 7:
                            nc.gpsimd.tensor_add(sl[:, 0, :], sl[:, 0, :],
                                                 sl[:, 1, :])
                # gpsimd also does the leading pair adds of the 8-15 chains
                elif h == 9:
                    sl2 = SLOW[(qb, gg)]
                    nc.gpsimd.tensor_add(sl2[:, 1, :], E[:, 8, :], E[:, 9, :])
                elif h == 13:
                    r2 = REC2[(qb, gg)]
                    nc.gpsimd.tensor_add(r2[:, 0, :], E[:, 12, :], E[:, 13, :])

            def emit_den_dve(qb, gg):
                """DVE: finish the 8-15 chains, D, recip, doubled rec,
                then the wave-ordered pair muls."""
                E = Egrp[(qb, gg)]
                sl = SLOW[(qb, gg)]
                rec2 = REC2[(qb, gg)]
                ud0 = sl[:, 1, :]       # heads 8-11 chain (seeded on GpSimd)
                ud1 = rec2[:, 0, :]     # heads 12-15 chain (seeded on GpSimd)
                nc.vector.tensor_add(ud0, ud0, E[:, 10, :])
                nc.vector.tensor_add(ud0, ud0, E[:, 11, :])
                nc.vector.tensor_add(ud1, ud1, E[:, 14, :])
                nc.vector.tensor_add(ud1, ud1, E[:, 15, :])
                nc.vector.tensor_add(ud0, ud0, ud1)
                denf = r_p.tile([128, 4 * QB], F32, tag="denf",
                                name=f"denf{qb}_{gg}")
                nc.vector.tensor_add(denf, ud0, sl[:, 0, :])
                nc.vector.reciprocal_approx_fast(out=denf, in_=denf)
                nc.vector.tensor_copy(rec2[:, 0, :], denf)
                nc.vector.tensor_copy(rec2[:, 1, :], denf)
                # wave-ordered pair muls, flattened to 2D contiguous APs so
                # the DVE picks the 2x_1P packed mode (3D APs fall to 1x)
                rf = rec2.rearrange("p a b -> p (a b)")
                for w in range(NPAIR):
                    ef = E[:, 2 * w:2 * w + 2, :].rearrange("p a b -> p (a b)")
                    nc.vector.tensor_mul(ef, ef, rf)

            def emit_attnv_wave(qb, w, outT):
                """attnV for head pair w over ALL 8 k-tiles, then one copy."""
                aw = ps_a.tile([128, 512], F32, tag="acc", name=f"aw{qb}_{w}")
                for kt in range(NKT):
                    E = Egrp[(qb, kt // 4)]
                    j = kt % 4
                    for i in range(2):
                        h = 2 * w + i
                        po = 64 * (h % 2)
                        nc.tensor.matmul(
                            aw[po:po + 64, 0:QB],
                            Vb[:, kt, h * HD:(h + 1) * HD],
                            E[:, h, j * QB:(j + 1) * QB],
                            start=(kt == 0), stop=(kt == NKT - 1),
                            tile_position=(0, po))
                nc.vector.tensor_copy(outT[:, w, :], aw[:, 0:QB])

            def emit_out_proj_qsub(qb, outT, qsub, wout):
                q0 = qb * QB
                psy = [ps_a.tile([128, 512], F32, tag="acc",
                                 name=f"psy{qb}_{qsub}_{ec}") for ec in range(2)]
                for ft in range(NCT):
                    lhsT = outT[:, ft, qsub * 128:(qsub + 1) * 128]
                    for ec in range(2):
                        nc.tensor.matmul(psy[ec], lhsT,
                                         wout[:, ft, ec * 512:(ec + 1) * 512],
                                         start=(ft == 0), stop=(ft == NCT - 1))
                for ec in range(2):
                    y = y_p.tile([128, 512], F32, tag="y",
                                 name=f"y{qb}_{qsub}_{ec}")
                    nc.vector.scalar_tensor_tensor(
                        y, psy[ec], 0.0, b_bcast[:, ec * 512:(ec + 1) * 512],
                        Bypass, Add)
                    nc.sync.dma_start(
                        out_ext[q0 + qsub * 128:q0 + (qsub + 1) * 128,
                                ec * 512:(ec + 1) * 512], y)

            # ---------- phase A: v_proj, then qk pairs + qb0 scores ----------
            new_group(0, 0)
            new_group(0, 1)
            with nc.named_scope("v_proj"):
                for st in range(NKT):
                    emit_v_st(st)
            with nc.named_scope("qk_attn_overlap"):
                for p in range(NPAIR):
                    emit_qk_ft(8 + p)   # K pair p
                    emit_qk_ft(p)       # Q pair p
                    # scores for the previous pair (1-pair lag keeps PE fed)
                    if p >= 1:
                        for gg in range(2):
                            for i in range(2):
                                emit_scores(0, gg, 2 * (p - 1) + i)
                for gg in range(2):
                    for i in range(2):
                        emit_scores(0, gg, 2 * 7 + i)
                emit_den_dve(0, 0)
                emit_den_dve(0, 1)

            # wout: reuses the xT pool slot (xT dead after qk_proj)
            wout = xt_p.tile([128, NCT, C], BF16, tag="xt", name="wout")
            for ct in range(NCT):
                nc.gpsimd.dma_start(wout[:, ct, :],
                                    wout_ext[ct * 128:(ct + 1) * 128, :])

            # outT scratch: reuses the wv slot (dead after v_proj); even/odd
            # qbs use disjoint column ranges, range-level deps handle reuse.
            scratch = w_p.tile([128, NCT, C], BF16, tag="wv", name="scratch")
            outT_views = [scratch[:, :, 0:QB], scratch[:, :, QB:2 * QB]]

            # ---------- phase B: attention pipeline over qbs ----------
            for qb in range(NQB):
                with nc.named_scope(f"attn_qb{qb}"):
                    outT = outT_views[qb % 2]
                    # next-group scores first: ACT runs ahead into (qb+1, 0)
                    if qb + 1 < NQB:
                        new_group(qb + 1, 0)
                        for h in range(H):
                            emit_scores(qb + 1, 0, h)
                    for w in range(NPAIR):
                        emit_attnv_wave(qb, w, outT)
                    # E(qb,*) free -> scores for (qb+1, 1)
                    if qb + 1 < NQB:
                        new_group(qb + 1, 1)
                        for h in range(H):
                            emit_scores(qb + 1, 1, h)
                    emit_out_proj_qsub(qb, outT, 0, wout)
                    emit_out_proj_qsub(qb, outT, 1, wout)
                    # den/mul work for the NEXT qb, emitted now so attnV(qb+1)
                    # is never mul-gated; these wait on (qb+1)'s exps, so they
                    # go LAST in the DVE queue.
                    if qb + 1 < NQB:
                        emit_den_dve(qb + 1, 0)
                        emit_den_dve(qb + 1, 1)

    nc.compile()
    return nc


_NC = None


def _get_nc():
    global _NC
    if _NC is None:
        _NC = build()
    return _NC


def kernel(x, w_qkv, w_out, b_out):
    nc = _get_nc()
    x = np.ascontiguousarray(np.asarray(x, dtype=np.float32))
    w_qkv = np.ascontiguousarray(np.asarray(w_qkv, dtype=np.float32))
    w_out = np.ascontiguousarray(np.asarray(w_out, dtype=np.float32))
    b_out = np.ascontiguousarray(np.asarray(b_out, dtype=np.float32))
    in_maps = [
        {"x": x[i], "w_qkv": w_qkv, "w_out": w_out, "b_out": b_out}
        for i in range(8)
    ]
    res = run_bass_kernel_spmd(nc, in_maps, core_ids=list(range(8)))
    out = np.stack([np.asarray(res.results[i]["out"]) for i in range(8)])
    return out.astype(np.float32)


# revision 23
# speedup vs baseline: 1.2830x; 1.2509x over previous
"""Distributed Trainium2 kernel for nn_Attention_21990232555717.

Reference (per batch element a, seq s=1024, model dim c=1024, 16 heads):
    qkv = x @ w_qkv                       # (s, 3072)
    q,k,v split per head (hd=64)
    scores = q @ k.T * (1/sqrt(1024))     # (h, s, s)
    attn = softmax(scores, axis=HEADS)    # normalize across the 16 heads!
    out = attn @ v -> (s, 1024) @ w_out + b_out

Sharding: pure data parallel - batch (8) across 8 cores, weights replicated.

v2.1 design:
  * all-bf16 datapath (rel err ~5e-3 vs the 2e-2 gate)
  * weights arrive bf16 via SWDGE cast-DMA: wv first (v_proj runs before
    qk_proj so Vb is ready early), then wqk, then wout
  * the ACT exp stream (128 exps of [128,1024] ~ 147us) is the wall:
    scores for qb0 chase the qk pairs during the projections, and in the
    steady state each qb block interleaves next-group scores so ACT never
    idles.  ACT does ONLY exps + QKT/Vb psum copies.
  * DVE: den chains (accumulate heads 8-15), recip, normalize muls
    (pair-granular against a doubled rec buffer - real strides keep the
    2x_1P mode; a broadcast AP drops to 1x and doubles the cost), attnV
    psum->outT copies, y bias.  All den/mul work for qb is emitted one
    block EARLY so attnV(qb) is never mul-gated.
  * GpSimd: denominator partial sums over heads 0-7 + the leading pair
    adds of the 8-15 chains (own queue, off critical path).
  * attnV accumulates all 8 k-tiles directly in PSUM (no spill/merge) -
    legal because the E ring holds 3 groups: 2 in e_p plus the dead wqk
    slot re-viewed as the third buffer.
  * PSUM: ps_proj [128,1024] bufs=1 (transposes+projections, 2 banks),
    ps_sc [128,1024] bufs=1 (scores, 2 banks), ps_a [128,512] bufs=4
    (attnV waves + out-proj units + bias bcast, 4 banks).
  * SBUF aliasing: wqk slot -> 3rd E buffer; wv slot -> outT scratch;
    xT slot -> wout.
"""

import numpy as np

import concourse.bass as bass
import concourse.mybir as mybir
import concourse.tile as tile
from concourse import bacc
from concourse.bass_utils import run_bass_kernel_spmd
from concourse.masks import make_identity

F32 = mybir.dt.float32
BF16 = mybir.dt.bfloat16
Exp = mybir.ActivationFunctionType.Exp
Bypass = mybir.AluOpType.bypass
Add = mybir.AluOpType.add

S = 1024      # sequence length per core (batch element)
C = 1024      # model dim
H = 16        # heads
HD = 64       # head dim
SCALE = 1.0 / (C ** 0.5)
QB = 256      # q block size
NQB = S // QB          # 4 q blocks
NKT = S // 128         # 8 k tiles
NCT = C // 128         # 8 contraction tiles
NPAIR = 8              # head pairs


def build():
    nc = bacc.Bacc(None, target_bir_lowering=False)
    x_ext = nc.declare_dram_parameter("x", [S, C], F32, isOutput=False)
    wqkv_ext = nc.declare_dram_parameter("w_qkv", [C, 3 * C], F32, isOutput=False)
    wout_ext = nc.declare_dram_parameter("w_out", [C, C], F32, isOutput=False)
    b_ext = nc.declare_dram_parameter("b_out", [C], F32, isOutput=False)
    out_ext = nc.declare_dram_parameter("out", [S, C], F32, isOutput=True)

    with tile.TileContext(nc) as tc:
        with (
            tc.tile_pool(name="const_p", bufs=1) as const_p,
            tc.tile_pool(name="xb_p", bufs=2) as xb_p,
            tc.tile_pool(name="xt_p", bufs=1) as xt_p,      # xT, then wout
            tc.tile_pool(name="w_p", bufs=1) as w_p,        # wqk->E3, wv->outT
            tc.tile_pool(name="act_p", bufs=1) as act_p,
            tc.tile_pool(name="e_p", bufs=2) as e_p,
            tc.tile_pool(name="d_p", bufs=1) as d_p,
            tc.tile_pool(name="r_p", bufs=1) as r_p,
            tc.tile_pool(name="y_p", bufs=2) as y_p,
            tc.tile_pool(name="ps_j", bufs=2, space="PSUM") as ps_j,
            tc.tile_pool(name="ps_s", bufs=1, space="PSUM") as ps_s,
            tc.tile_pool(name="ps_a", bufs=2, space="PSUM") as ps_a,
        ):
            # ---- constants FIRST: make_identity uses the GpSimd queue and
            # must not sit behind the (slow, serialized) SWDGE weight DMAs
            ident = const_p.tile([128, 128], BF16)
            make_identity(nc, ident)
            ones1 = const_p.tile([1, 128], BF16)
            nc.vector.memset(ones1, 1.0)

            # ---- weights: SWDGE cast-DMA (serial ~2.5us per transfer) ----
            wqk = w_p.tile([128, NCT, 2 * C], BF16, tag="wqk", name="wqk")  # 32 KB
            wv = w_p.tile([128, NCT, C], BF16, tag="wv", name="wv")         # 16 KB
            for ct in range(NCT):
                nc.gpsimd.dma_start(wv[:, ct, :],
                                    wqkv_ext[ct * 128:(ct + 1) * 128, 2 * C:3 * C])
            for ct in range(NCT):
                nc.gpsimd.dma_start(wqk[:, ct, :],
                                    wqkv_ext[ct * 128:(ct + 1) * 128, 0:2 * C])
            b_sb = const_p.tile([1, C], BF16)
            nc.gpsimd.dma_start(b_sb, b_ext[None, :])

            # ---- persistent activations ----
            QKT = act_p.tile([128, H, S], BF16)        # 32 KB/part (ft 0-7 Q, 8-15 K)
            Vb = act_p.tile([128, NKT, C], BF16)       # 16 KB/part

            # ---- x: HWDGE f32 DMA staged INSIDE E(0,0) (f32 view; the tile
            # is not written by exps until ~60us) -> DVE cast -> PE transpose.
            # All 8 slab DMAs land in parallel (no ring-latency serialization).
            E00 = e_p.tile([128, H, 4 * QB], BF16, tag="E", name="E0_0")
            xstage = E00.bitcast(F32).rearrange("p a b -> p (a b)")  # [128, 8192] f32
            xT = xt_p.tile([128, NCT, S], BF16, tag="xt", name="xT")  # 16 KB
            for st in range(NKT):
                xf = xstage[:, st * C:(st + 1) * C]
                eng = nc.sync if st % 2 == 0 else nc.scalar
                eng.dma_start(xf, x_ext[st * 128:(st + 1) * 128, :])
                xb = xb_p.tile([128, C], BF16, tag="xb", name=f"xb{st}")
                nc.vector.tensor_copy(xb, xf)
                pt = ps_j.tile([128, S], F32, tag="pj", name=f"pt{st}")
                ptb = pt.bitcast(BF16)  # bf16 view: transpose out must be bf16
                for ct in range(NCT):
                    nc.tensor.transpose(ptb[:, ct * 128:(ct + 1) * 128],
                                        xb[:, ct * 128:(ct + 1) * 128], ident)
                ptv = ptb[:, 0:C].rearrange("p (a b) -> p a b", a=NCT)
                nc.vector.tensor_copy(xT[:, :, st * 128:(st + 1) * 128], ptv)

            # ================= interleaved main pipeline =================
            Egrp = {}      # (qb, gg) -> E tile view [128, H, 4*QB]
            SLOW = {}      # (qb, gg) -> partial-sum chains tile [128, 2, S]
            REC2 = {}      # (qb, gg) -> doubled-rec tile [128, 2, S] bf16
            # chain storage aliasing (saves SBUF):
            #   sl[:,0,:]  heads 0-3 then 0-7 sum  (GpSimd)
            #   sl[:,1,:]  heads 4-7 sum (GpSimd), then REUSED as the
            #              heads 8-11 chain (GpSimd pair add + DVE accums -
            #              same-queue order makes the WAR safe)
            #   rec2[:,0,:] heads 12-15 chain, overwritten by rec after its
            #              last read; rec2[:,1,:] = second rec copy

            def emit_qk_ft(ft):
                ps = ps_j.tile([128, S], F32, tag="pj", name=f"qk{ft}")
                for ct in range(NCT):
                    lhsT = wqk[:, ct, ft * 128:(ft + 1) * 128]
                    for sb in range(2):
                        nc.tensor.matmul(
                            ps[:, sb * 512:(sb + 1) * 512], lhsT,
                            xT[:, ct, sb * 512:(sb + 1) * 512],
                            start=(ct == 0), stop=(ct == NCT - 1))
                nc.scalar.copy(QKT[:, ft, :], ps)

            def emit_v_st(st):
                ps = ps_j.tile([128, S], F32, tag="pj", name=f"v{st}")
                for ct in range(NCT):
                    lhsT = xT[:, ct, st * 128:(st + 1) * 128]
                    for fb in range(2):
                        nc.tensor.matmul(
                            ps[:, fb * 512:(fb + 1) * 512], lhsT,
                            wv[:, ct, fb * 512:(fb + 1) * 512],
                            start=(ct == 0), stop=(ct == NCT - 1))
                nc.scalar.copy(Vb[:, st, :], ps)

            def new_group(qb, gg):
                i = 2 * qb + gg
                if i == 0:
                    Egrp[(qb, gg)] = E00  # pre-allocated (doubles as x stage)
                elif i % 3 == 2:
                    # 3rd ring slot: re-view the dead wqk slot (same bytes)
                    ew = w_p.tile([128, NCT, 2 * C], BF16, tag="wqk",
                                  name=f"Ew{i}")
                    Egrp[(qb, gg)] = ew.rearrange("p a (b c) -> p (a b) c",
                                                  b=2, c=1024)
                else:
                    Egrp[(qb, gg)] = e_p.tile([128, H, 4 * QB], BF16, tag="E",
                                              name=f"E{qb}_{gg}")
                SLOW[(qb, gg)] = d_p.tile([128, 2, 4 * QB], BF16, tag="slow",
                                          name=f"sl{qb}_{gg}")
                REC2[(qb, gg)] = r_p.tile([128, 2, 4 * QB], BF16, tag="rec2",
                                          name=f"rec2{qb}_{gg}")

            def emit_scores(qb, gg, h):
                """scores + exp + gpsimd denominator links for head h."""
                E = Egrp[(qb, gg)]
                sl = SLOW[(qb, gg)]
                q0 = qb * QB
                po = 64 * (h % 2)
                rhs = QKT[po:po + 64, h // 2, q0:q0 + QB]
                pss = ps_s.tile([128, 4 * QB], F32, tag="ps",
                                name=f"sc{qb}_{gg}_{h}")
                for j in range(4):
                    kt = 4 * gg + j
                    lhsT = QKT[po:po + 64, 8 + h // 2, kt * 128:(kt + 1) * 128]
                    nc.tensor.matmul(pss[:, j * QB:(j + 1) * QB], lhsT, rhs,
                                     start=True, stop=True)
                nc.scalar.activation(E[:, h, :], pss, Exp, scale=SCALE)
                # qb0 only: gpsimd denominator tree over heads 0-7 (phase A,
                # where DVE is idle).  GpSimd SBUF traffic degrades concurrent
                # DVE throughput ~2.5x, so it must NOT run in phase B.
                if qb == 0:
                    if h < 8 and h % 2 == 1:
                        c = h // 4
                        if h % 4 == 1:
                            nc.gpsimd.tensor_add(sl[:, c, :], E[:, h - 1, :],
                                                 E[:, h, :])
                        else:
                            nc.gpsimd.tensor_add(sl[:, c, :], sl[:, c, :],
                                                 E[:, h - 1, :])
                            nc.gpsimd.tensor_add(sl[:, c, :], sl[:, c, :],
                                                 E[:, h, :])
                            if h == 7:
                                nc.gpsimd.tensor_add(sl[:, 0, :], sl[:, 0, :],
                                                     sl[:, 1, :])
                    elif h == 9:
                        nc.gpsimd.tensor_add(sl[:, 1, :], E[:, 8, :], E[:, 9, :])
                    elif h == 13:
                        r2 = REC2[(qb, gg)]
                        nc.gpsimd.tensor_add(r2[:, 0, :], E[:, 12, :], E[:, 13, :])

            def emit_den_dve(qb, gg):
                """DVE: finish (qb0) or fully build (qb>0) the denominator,
                recip, doubled rec, then the wave-ordered pair muls."""
                E = Egrp[(qb, gg)]
                sl = SLOW[(qb, gg)]
                rec2 = REC2[(qb, gg)]
                ud0 = sl[:, 1, :]
                ud1 = rec2[:, 0, :]
                if qb == 0:
                    # heads 0-7 summed on gpsimd into sl0; 8-9/12-13 seeded
                    nc.vector.tensor_add(ud0, ud0, E[:, 10, :])
                    nc.vector.tensor_add(ud0, ud0, E[:, 11, :])
                    nc.vector.tensor_add(ud1, ud1, E[:, 14, :])
                    nc.vector.tensor_add(ud1, ud1, E[:, 15, :])
                    nc.vector.tensor_add(ud0, ud0, ud1)
                else:
                    # all-DVE: three ~balanced chains into sl0 / ud0 / ud1
                    sl0 = sl[:, 0, :]
                    nc.vector.tensor_add(sl0, E[:, 0, :], E[:, 1, :])
                    for h in (2, 3, 4):
                        nc.vector.tensor_add(sl0, sl0, E[:, h, :])
                    nc.vector.tensor_add(ud0, E[:, 5, :], E[:, 6, :])
                    for h in (7, 8, 9):
                        nc.vector.tensor_add(ud0, ud0, E[:, h, :])
                    nc.vector.tensor_add(ud1, E[:, 10, :], E[:, 11, :])
                    for h in (12, 13, 14, 15):
                        nc.vector.tensor_add(ud1, ud1, E[:, h, :])
                    nc.vector.tensor_add(ud0, ud0, ud1)
                denf = r_p.tile([128, 4 * QB], F32, tag="denf",
                                name=f"denf{qb}_{gg}")
                nc.vector.tensor_add(denf, ud0, sl[:, 0, :])
                nc.vector.reciprocal_approx_fast(out=denf, in_=denf)
                nc.vector.tensor_copy(rec2[:, 0, :], denf)
                nc.vector.tensor_copy(rec2[:, 1, :], denf)
                # wave-ordered pair muls, flattened to 2D contiguous APs so
                # the DVE picks the 2x_1P packed mode (3D APs fall to 1x)
                rf = rec2.rearrange("p a b -> p (a b)")
                for w in range(NPAIR):
                    ef = E[:, 2 * w:2 * w + 2, :].rearrange("p a b -> p (a b)")
                    nc.vector.tensor_mul(ef, ef, rf)

            def emit_attnv_wave(qb, w, outT):
                """attnV for head pair w over ALL 8 k-tiles, then one copy."""
                aw = ps_a.tile([128, 512], F32, tag="acc", name=f"aw{qb}_{w}")
                for kt in range(NKT):
                    E = Egrp[(qb, kt // 4)]
                    j = kt % 4
                    for i in range(2):
                        h = 2 * w + i
                        po = 64 * (h % 2)
                        nc.tensor.matmul(
                            aw[po:po + 64, 0:QB],
                            Vb[:, kt, h * HD:(h + 1) * HD],
                            E[:, h, j * QB:(j + 1) * QB],
                            start=(kt == 0), stop=(kt == NKT - 1),
                            tile_position=(0, po))
                nc.scalar.copy(outT[:, w, :], aw[:, 0:QB])

            def emit_out_proj_qsub(qb, outT, qsub, wout):
                q0 = qb * QB
                psy = [ps_a.tile([128, 512], F32, tag="acc",
                                 name=f"psy{qb}_{qsub}_{ec}") for ec in range(2)]
                for ft in range(NCT):
                    lhsT = outT[:, ft, qsub * 128:(qsub + 1) * 128]
                    for ec in range(2):
                        nc.tensor.matmul(psy[ec], lhsT,
                                         wout[:, ft, ec * 512:(ec + 1) * 512],
                                         start=(ft == 0), stop=False)
                for ec in range(2):
                    # bias via a K=1 ones-row matmul into the same psum group
                    nc.tensor.matmul(psy[ec], ones1,
                                     b_sb[:, ec * 512:(ec + 1) * 512],
                                     start=False, stop=True)
                    y = y_p.tile([128, 512], F32, tag="y",
                                 name=f"y{qb}_{qsub}_{ec}")
                    nc.scalar.copy(y, psy[ec])
                    nc.sync.dma_start(
                        out_ext[q0 + qsub * 128:q0 + (qsub + 1) * 128,
                                ec * 512:(ec + 1) * 512], y)

            # ---------- phase A: v_proj, then qk pairs + qb0 scores ----------
            new_group(0, 0)
            new_group(0, 1)
            with nc.named_scope("v_proj"):
                for st in range(NKT):
                    emit_v_st(st)
            with nc.named_scope("qk_attn_overlap"):
                for p in range(NPAIR):
                    emit_qk_ft(8 + p)   # K pair p
                    emit_qk_ft(p)       # Q pair p
                    # scores for the previous pair (1-pair lag keeps PE fed)
                    if p >= 1:
                        for gg in range(2):
                            for i in range(2):
                                emit_scores(0, gg, 2 * (p - 1) + i)
                for gg in range(2):
                    for i in range(2):
                        emit_scores(0, gg, 2 * 7 + i)
                emit_den_dve(0, 0)
                emit_den_dve(0, 1)

            # wout: reuses the xT pool slot (xT dead after qk_proj)
            wout = xt_p.tile([128, NCT, C], BF16, tag="xt", name="wout")
            for ct in range(NCT):
                nc.gpsimd.dma_start(wout[:, ct, :],
                                    wout_ext[ct * 128:(ct + 1) * 128, :])

            # outT scratch: reuses the wv slot (dead after v_proj); even/odd
            # qbs use disjoint column ranges, range-level deps handle reuse.
            scratch = w_p.tile([128, NCT, C], BF16, tag="wv", name="scratch")
            outT_views = [scratch[:, :, 0:QB], scratch[:, :, QB:2 * QB]]

            # ---------- phase B: attention pipeline over qbs ----------
            for qb in range(NQB):
                with nc.named_scope(f"attn_qb{qb}"):
                    outT = outT_views[qb % 2]
                    # next-group scores first: ACT runs ahead into (qb+1, 0)
                    if qb + 1 < NQB:
                        new_group(qb + 1, 0)
                        for h in range(H):
                            emit_scores(qb + 1, 0, h)
                    for w in range(NPAIR):
                        emit_attnv_wave(qb, w, outT)
                    # E(qb,*) free -> scores for (qb+1, 1)
                    if qb + 1 < NQB:
                        new_group(qb + 1, 1)
                        for h in range(H):
                            emit_scores(qb + 1, 1, h)
                    emit_out_proj_qsub(qb, outT, 0, wout)
                    emit_out_proj_qsub(qb, outT, 1, wout)
                    # den/mul work for the NEXT qb, emitted now so attnV(qb+1)
                    # is never mul-gated; these wait on (qb+1)'s exps, so they
                    # go LAST in the DVE queue.
                    if qb + 1 < NQB:
                        emit_den_dve(qb + 1, 0)
                        emit_den_dve(qb + 1, 1)

    nc.compile()
    return nc


_NC = None


def _get_nc():
    global _NC
    if _NC is None:
        _NC = build()
    return _NC


def kernel(x, w_qkv, w_out, b_out):
    nc = _get_nc()
    x = np.ascontiguousarray(np.asarray(x, dtype=np.float32))
    w_qkv = np.ascontiguousarray(np.asarray(w_qkv, dtype=np.float32))
    w_out = np.ascontiguousarray(np.asarray(w_out, dtype=np.float32))
    b_out = np.ascontiguousarray(np.asarray(b_out, dtype=np.float32))
    in_maps = [
        {"x": x[i], "w_qkv": w_qkv, "w_out": w_out, "b_out": b_out}
        for i in range(8)
    ]
    res = run_bass_kernel_spmd(nc, in_maps, core_ids=list(range(8)))
    out = np.stack([np.asarray(res.results[i]["out"]) for i in range(8)])
    return out.astype(np.float32)
